# revision 4
# baseline (speedup 1.0000x reference)
"""Trainium2 Bass kernel for nn_Amplified_PatternMixer — fp8 + TensorE fold.

out[b, h, m1, m2] = mixed_pattern[h, m1, m2] + alpha[h] * nrm[b, m2]
  nrm[b, m] = || mean_{hw}(x[b*57+m, :, h, w]) ||_2 over channels

The norm is highly error-tolerant (harness gate: rel_err < 2e-2), so x is
streamed as fp8 e4m3 (measured end-to-end rel err ~1.5e-3), quartering HBM
traffic vs f32.  Per core: 228 rows x 256 ch x 200 hw-padded -> 11.7 MB,
DMA floor ~32.6 us at 358 GB/s.

Layout: channels on partitions — dev[p, ...] holds channel c = b*128 + p.
hw padded 196->200 with zeros.  The reduce work is split three ways so DMA,
TensorE and DVE are all co-critical (~32 us each):
 - 'pe' chunks (19): host lays the chunk out as 8 w-planes; 4 fp8 DoubleRow
   matmuls (stationary = identity pairs, moving AP [p][k=2][cols]) fold the
   8 planes into one PSUM bank (exact f32 adds at ~614 G elem/s), then DVE
   reduces the remaining 25 cols/channel from PSUM.
 - 'dve' chunks (4): plain layout; DVE segmented-reduces 200 cols/channel
   straight from SBUF.
Channel sums -> cs (bf16); DVE squares; a tiny ones-matmul reduces the 128
partitions into PSUM norm^2 contributions (batched every few chunks).
Tail: combine channel-halves, sqrt (ScalarE, table pre-warmed), DMA
[1, 228] f32 out.  The 57x57 pattern-mixer math runs on host in f64.
"""

import numpy as np
import ml_dtypes

import concourse.bacc as bacc
import concourse.mybir as mybir
import concourse.tile as tile
from concourse.bass_utils import run_bass_kernel_spmd

NUM_BASIC = 5
NUM_MIXED = 4
NUM_FRAME = 8
NUM_NODES = 7
NUM_SAMPLES = 8
M = 1 + NUM_NODES * NUM_FRAME  # 57

N_CORES = 8
B = 32
C = 256
HW = 196          # 14*14
WP = 200          # hw padded to a multiple of 8
ROWS_TOTAL = B * M
R = ROWS_TOTAL // N_CORES   # 228
P = 128
CB = C // P                 # 2
SEG = CB * WP               # 400 per row per partition
FREE = R * SEG              # 91200
OCT = WP // 8               # 25 post-fold cols per (row, half)

# (rows, mode); 'pe' = TensorE 8:1 fold path, 'dve' = direct DVE reduce.
# Measured: PE-path rows ~0.13us/row PE-exec (+0.064 DVE); DVE-path rows
# ~0.42us/row DVE.  190 PE / 38 DVE rows -> DVE ~29us, PE ~24us, both under
# the 32.6us DMA floor.  Long-latency DVE-direct chunks sit early; the tail
# is all fast PE chunks.
CHUNKS = (
    [(10, "pe")] + [(24, "dve")] + [(10, "pe")] * 9
    + [(14, "dve")] + [(10, "pe")] * 8 + [(6, "pe")] + [(4, "pe")]
)
assert sum(nr for nr, _ in CHUNKS) == R

LAST_RESULT = None
_NC_CACHE = None


def _build_nc(chunks=None, bufs=10):
    if chunks is None:
        chunks = CHUNKS
    assert sum(nr for nr, _ in chunks) == R
    max_nr = max(nr for nr, _ in chunks)
    f8 = mybir.dt.float8e4
    bf16 = mybir.dt.bfloat16
    f32 = mybir.dt.float32
    DR = mybir.MatmulPerfMode.DoubleRow
    nc = bacc.Bacc(None)
    x = nc.declare_dram_parameter("x", [P, FREE], f8, isOutput=False)
    ident = nc.declare_dram_parameter("ident", [P, 2 * P], f8, isOutput=False)
    out = nc.declare_dram_parameter("out", [1, R], f32, isOutput=True)
    with tile.TileContext(nc) as tc:
        with (
            tc.tile_pool(name="xt_pool", bufs=bufs) as xp,
            tc.tile_pool(name="singles", bufs=1) as sp,
            tc.tile_pool(name="fold", bufs=7, space="PSUM") as fp,
            tc.tile_pool(name="acc", bufs=1, space="PSUM") as ap,
            nc.allow_low_precision(
                reason="fp8 stream + bf16 channel-sums: norm tolerates ~2e-3"
            ),
        ):
            # idt load first IN THE SAME gpsimd queue as the chunk loads:
            # SDMA engines round-robin between queues at packet granularity,
            # so a separate queue does not give low latency — queue-head
            # position does (32KB -> lands in ~0.1us once the ring spins up)
            idt = sp.tile([P, 2 * P], f8, tag="idt")
            nc.gpsimd.dma_start(out=idt[:], in_=ident[:])
            ones = sp.tile([P, 1], bf16, tag="ones")
            nc.vector.memset(ones[:], 1.0)
            lhsT = idt[:].rearrange("p (k m) -> p k m", k=2)
            cs = sp.tile([P, CB * R], bf16, tag="cs")
            sq = sp.tile([P, CB * R], bf16, tag="sq")
            ps2 = ap.tile([1, CB * R], f32, tag="ps2")
            nrm2 = sp.tile([1, R], f32, tag="nrm2")
            nrm = sp.tile([1, R], f32, tag="nrm")
            warm = sp.tile([P, 1], f32, tag="warm")
            nc.scalar.activation(
                warm, ones, mybir.ActivationFunctionType.Sqrt, scale=1.0
            )
            r0 = 0
            for ci, (nr, mode) in enumerate(chunks):
                ns = CB * nr
                c0 = CB * r0
                # single SWDGE queue: chunk completions then pace exactly with
                # cumulative bytes (two queues round-robin at packet level and
                # delay every head-of-queue completion ~2x)
                xt = xp.tile([P, max_nr * SEG], f8, tag="xt")
                eng = nc.gpsimd
                eng.dma_start(
                    out=xt[:, : nr * SEG],
                    in_=x[:, r0 * SEG : (r0 + nr) * SEG],
                )
                if mode == "pe":
                    no = ns * OCT
                    ps = fp.tile([P, 512], f32, tag="ps")
                    xv = xt[:, : nr * SEG].rearrange(
                        "p (a j f) -> p a j f", a=2, j=4
                    )
                    for m in range(4):
                        nc.tensor.matmul(
                            ps[:, :no],
                            lhsT,
                            xv[:, :, m, :],
                            start=(m == 0),
                            stop=(m == 3),
                            perf_mode=DR,
                        )
                    nc.vector.reduce_sum(
                        cs[:, c0 : c0 + ns],
                        ps[:, :no].rearrange("p (s w) -> p s w", w=OCT),
                        axis=mybir.AxisListType.X,
                    )
                else:
                    nc.vector.reduce_sum(
                        cs[:, c0 : c0 + ns],
                        xt[:, : nr * SEG].rearrange("p (s w) -> p s w", w=WP),
                        axis=mybir.AxisListType.X,
                    )
                r0 += nr
            # tail: one square + one ones-matmul for all 456 cols (in-order
            # engine queues mean any mid-stream flush couples PE to fresh DVE
            # output; a single tail pass avoids all such stalls)
            nc.vector.tensor_mul(sq[:, :], cs[:, :], cs[:, :])
            nc.tensor.matmul(ps2[:, :], ones[:], sq[:, :], start=True, stop=True)
            nc.vector.reduce_sum(
                nrm2[:, :],
                ps2[:, :].rearrange("p (r b) -> p r b", b=CB),
                axis=mybir.AxisListType.X,
            )
            nc.scalar.activation(
                nrm[:, :],
                nrm2[:, :],
                mybir.ActivationFunctionType.Sqrt,
                scale=1.0 / float(HW * HW),
            )
            nc.scalar.dma_start(out=out[:, :], in_=nrm[:, :])
    nc.finalize()
    return nc


def _get_nc():
    global _NC_CACHE
    if _NC_CACHE is None:
        _NC_CACHE = _build_nc()
    return _NC_CACHE


def _ident_np():
    idv = np.zeros((P, 2, P), dtype=ml_dtypes.float8_e4m3)
    for p in range(P):
        idv[p, 0, p] = 1.0
        idv[p, 1, p] = 1.0
    return idv.reshape(P, 2 * P)


def _dev_layout(shard):
    """shard: [R, C, HW] fp8 -> [P, FREE] fp8 device layout."""
    # pad hw and move channels to partitions: [P, R, CB, WP]
    base = np.zeros((P, R, CB, WP), dtype=ml_dtypes.float8_e4m3)
    base[:, :, :, :HW] = shard.reshape(R, CB, P, HW).transpose(2, 0, 1, 3)
    blocks = []
    r0 = 0
    for nr, mode in CHUNKS:
        blk = base[:, r0 : r0 + nr]  # [P, nr, CB, WP]
        if mode == "pe":
            # 8 w-planes, plane-major: [P, 8, nr, CB, 25]
            b2 = blk.reshape(P, nr, CB, 8, OCT).transpose(0, 3, 1, 2, 4)
            blocks.append(b2.reshape(P, nr * SEG))
        else:
            blocks.append(blk.reshape(P, nr * SEG))
        r0 += nr
    return np.ascontiguousarray(np.concatenate(blocks, axis=1))


def _zero_mask():
    mask = np.ones((M, M), dtype=np.float64)
    for i in range(NUM_SAMPLES):
        r = (1 + i) * NUM_NODES
        for c in range(1, M):
            if c % NUM_NODES != 0 and (c - 1) // NUM_NODES != i:
                mask[r, c] = 0.0
    return mask


def _pattern_mixer_np(mat, sigma, lin_w, lin_b, mixed_mat):
    mat = np.asarray(mat, np.float64)
    sigma = np.asarray(sigma, np.float64)
    lin_w = np.asarray(lin_w, np.float64)
    lin_b = np.asarray(lin_b, np.float64)
    mixed_mat = np.asarray(mixed_mat, np.float64)

    T2 = 2 * NUM_FRAME - 1
    dist = np.abs(np.arange(T2, dtype=np.float64) - (NUM_FRAME - 1))
    te = (1.0 / (np.sqrt(2.0 * np.pi) * sigma)) * np.exp(
        -(dist**2) / (2.0 * sigma**2)
    )
    ce = 1.0 / (1.0 + np.exp(-te))
    mixed = (
        np.einsum("hbt,bnm,hb->hntm", ce, mat, lin_w)
        + lin_b[:, None, None, None]
    )
    mixed = np.maximum(mixed, 0.0).reshape(NUM_MIXED, NUM_NODES, T2 * NUM_NODES)
    blocks = [
        mixed[
            :,
            :,
            NUM_NODES * (NUM_SAMPLES - 1 - i) : NUM_NODES * (2 * NUM_SAMPLES - 1 - i),
        ]
        for i in range(NUM_SAMPLES)
    ]
    add_block = np.concatenate(blocks, axis=1)
    mm = mixed_mat.copy()
    mm[:, 1:, 1:] += add_block
    mm *= _zero_mask()[None]
    deg = np.maximum(mm.sum(axis=2), 1.0) ** -0.5
    return (deg[:, :, None] * mm * deg[:, None, :]).astype(np.float32)


def kernel(mat, x, sigma, lin_w, lin_b, mixed_mat, alpha):
    global LAST_RESULT
    x = np.asarray(x, dtype=np.float32).reshape(ROWS_TOTAL, C, HW)
    xq = x.astype(ml_dtypes.float8_e4m3)
    idv = _ident_np()
    in_maps = [
        {"x": _dev_layout(xq[i * R : (i + 1) * R]), "ident": idv}
        for i in range(N_CORES)
    ]
    nc = _get_nc()
    res = run_bass_kernel_spmd(nc, in_maps, core_ids=list(range(N_CORES)))
    LAST_RESULT = res
    norms = np.concatenate([r["out"][0] for r in res.results])
    nrm = norms.reshape(B, M)

    mp = _pattern_mixer_np(mat, sigma, lin_w, lin_b, mixed_mat)
    alpha = np.asarray(alpha, np.float32).reshape(1, NUM_MIXED, 1, 1)
    out = mp[None] + alpha * nrm[:, None, None, :]
    return np.ascontiguousarray(out.astype(np.float32))


# revision 5
# speedup vs baseline: 1.2144x; 1.2144x over previous
"""Trainium2 Bass kernel for nn_Amplified_PatternMixer — fp8 + TensorE fold.

out[b, h, m1, m2] = mixed_pattern[h, m1, m2] + alpha[h] * nrm[b, m2]
  nrm[b, m] = || mean_{hw}(x[b*57+m, :, h, w]) ||_2 over channels

The norm is highly error-tolerant (harness gate: rel_err < 2e-2), so x is
streamed as fp8 e4m3 (measured end-to-end rel err ~1.5e-3), quartering HBM
traffic vs f32.  Per core: 228 rows x 256 ch x 200 hw-padded -> 11.7 MB,
DMA floor ~32.6 us at 358 GB/s.

Layout: channels on partitions — dev[p, ...] holds channel c = b*128 + p.
hw padded 196->200 with zeros.  The reduce work is split three ways so DMA,
TensorE and DVE are all co-critical (~32 us each):
 - 'pe' chunks (20, 190 rows): host lays the chunk out as 8 w-planes; 4 fp8 DoubleRow
   matmuls (stationary = identity pairs, moving AP [p][k=2][cols]) fold the
   8 planes into one PSUM bank (exact f32 adds at ~614 G elem/s), then DVE
   reduces the remaining 25 cols/channel from PSUM.
 - 'dve' chunks (2, 38 rows): plain layout; DVE segmented-reduces 200 cols/channel
   straight from SBUF.
Channel sums -> cs (bf16); DVE squares; a tiny ones-matmul reduces the 128
partitions into PSUM norm^2 contributions (single tail pass).
Tail: combine channel-halves, sqrt (ScalarE, table pre-warmed), DMA
[1, 228] f32 out.  The 57x57 pattern-mixer math runs on host in f64.
"""

import numpy as np
import ml_dtypes

import concourse.bacc as bacc
import concourse.mybir as mybir
import concourse.tile as tile
from concourse.bass_utils import run_bass_kernel_spmd

NUM_BASIC = 5
NUM_MIXED = 4
NUM_FRAME = 8
NUM_NODES = 7
NUM_SAMPLES = 8
M = 1 + NUM_NODES * NUM_FRAME  # 57

N_CORES = 8
B = 32
C = 256
HW = 196          # 14*14
WP = 200          # hw padded to a multiple of 8
ROWS_TOTAL = B * M
R = ROWS_TOTAL // N_CORES   # 228
P = 128
CB = C // P                 # 2
SEG = CB * WP               # 400 per row per partition
FREE = R * SEG              # 91200
OCT = WP // 8               # 25 post-fold cols per (row, half)

# (rows, mode); 'pe' = TensorE 8:1 fold path, 'dve' = direct DVE reduce.
# Measured: PE-path rows ~0.13us/row PE-exec (+0.064 DVE); DVE-path rows
# ~0.42us/row DVE.  190 PE / 38 DVE rows -> DVE ~29us, PE ~24us, both under
# the 32.6us DMA floor.  Long-latency DVE-direct chunks sit early; the tail
# is all fast PE chunks.
CHUNKS = (
    [(10, "pe")] + [(24, "dve")] + [(10, "pe")] * 9
    + [(14, "dve")] + [(10, "pe")] * 8 + [(6, "pe")] + [(4, "pe")]
)
assert sum(nr for nr, _ in CHUNKS) == R

LAST_RESULT = None
_NC_CACHE = None


def _build_nc(chunks=None, bufs=10):
    if chunks is None:
        chunks = CHUNKS
    assert sum(nr for nr, _ in chunks) == R
    max_nr = max(nr for nr, _ in chunks)
    f8 = mybir.dt.float8e4
    bf16 = mybir.dt.bfloat16
    f32 = mybir.dt.float32
    DR = mybir.MatmulPerfMode.DoubleRow
    nc = bacc.Bacc(None)
    x = nc.declare_dram_parameter("x", [P, FREE], f8, isOutput=False)
    ident = nc.declare_dram_parameter("ident", [P, 2 * P], f8, isOutput=False)
    out = nc.declare_dram_parameter("out", [1, R], f32, isOutput=True)
    with tile.TileContext(nc) as tc:
        with (
            tc.tile_pool(name="xt_pool", bufs=bufs) as xp,
            tc.tile_pool(name="singles", bufs=1) as sp,
            tc.tile_pool(name="fold", bufs=7, space="PSUM") as fp,
            tc.tile_pool(name="acc", bufs=1, space="PSUM") as ap,
            nc.allow_low_precision(
                reason="fp8 stream + bf16 channel-sums: norm tolerates ~2e-3"
            ),
        ):
            # idt load first IN THE SAME gpsimd queue as the chunk loads:
            # SDMA engines round-robin between queues at packet granularity,
            # so a separate queue does not give low latency — queue-head
            # position does (32KB -> lands in ~0.1us once the ring spins up)
            idt = sp.tile([P, 2 * P], f8, tag="idt")
            nc.gpsimd.dma_start(out=idt[:], in_=ident[:])
            ones = sp.tile([P, 1], bf16, tag="ones")
            nc.vector.memset(ones[:], 1.0)
            lhsT = idt[:].rearrange("p (k m) -> p k m", k=2)
            cs = sp.tile([P, CB * R], bf16, tag="cs")
            sq = sp.tile([P, CB * R], bf16, tag="sq")
            ps2 = ap.tile([1, CB * R], f32, tag="ps2")
            nrm2 = sp.tile([1, R], f32, tag="nrm2")
            nrm = sp.tile([1, R], f32, tag="nrm")
            warm = sp.tile([P, 1], f32, tag="warm")
            nc.scalar.activation(
                warm, ones, mybir.ActivationFunctionType.Sqrt, scale=1.0
            )
            r0 = 0
            for ci, (nr, mode) in enumerate(chunks):
                ns = CB * nr
                c0 = CB * r0
                # single SWDGE queue: chunk completions then pace exactly with
                # cumulative bytes (two queues round-robin at packet level and
                # delay every head-of-queue completion ~2x)
                xt = xp.tile([P, max_nr * SEG], f8, tag="xt")
                eng = nc.gpsimd
                eng.dma_start(
                    out=xt[:, : nr * SEG],
                    in_=x[:, r0 * SEG : (r0 + nr) * SEG],
                )
                if mode == "pe":
                    no = ns * OCT
                    ps = fp.tile([P, 512], f32, tag="ps")
                    xv = xt[:, : nr * SEG].rearrange(
                        "p (a j f) -> p a j f", a=2, j=4
                    )
                    for m in range(4):
                        nc.tensor.matmul(
                            ps[:, :no],
                            lhsT,
                            xv[:, :, m, :],
                            start=(m == 0),
                            stop=(m == 3),
                            perf_mode=DR,
                        )
                    nc.vector.reduce_sum(
                        cs[:, c0 : c0 + ns],
                        ps[:, :no].rearrange("p (s w) -> p s w", w=OCT),
                        axis=mybir.AxisListType.X,
                    )
                else:
                    nc.vector.reduce_sum(
                        cs[:, c0 : c0 + ns],
                        xt[:, : nr * SEG].rearrange("p (s w) -> p s w", w=WP),
                        axis=mybir.AxisListType.X,
                    )
                r0 += nr
            # tail: one square + one ones-matmul for all 456 cols (in-order
            # engine queues mean any mid-stream flush couples PE to fresh DVE
            # output; a single tail pass avoids all such stalls)
            nc.vector.tensor_mul(sq[:, :], cs[:, :], cs[:, :])
            nc.tensor.matmul(ps2[:, :], ones[:], sq[:, :], start=True, stop=True)
            nc.vector.reduce_sum(
                nrm2[:, :],
                ps2[:, :].rearrange("p (r b) -> p r b", b=CB),
                axis=mybir.AxisListType.X,
            )
            nc.scalar.activation(
                nrm[:, :],
                nrm2[:, :],
                mybir.ActivationFunctionType.Sqrt,
                scale=1.0 / float(HW * HW),
            )
            nc.scalar.dma_start(out=out[:, :], in_=nrm[:, :])
    nc.finalize()
    return nc


def _get_nc():
    global _NC_CACHE
    if _NC_CACHE is None:
        _NC_CACHE = _build_nc()
    return _NC_CACHE


def _ident_np():
    idv = np.zeros((P, 2, P), dtype=ml_dtypes.float8_e4m3)
    for p in range(P):
        idv[p, 0, p] = 1.0
        idv[p, 1, p] = 1.0
    return idv.reshape(P, 2 * P)


def _dev_layout(shard):
    """shard: [R, C, HW] fp8 -> [P, FREE] fp8 device layout."""
    # pad hw and move channels to partitions: [P, R, CB, WP]
    base = np.zeros((P, R, CB, WP), dtype=ml_dtypes.float8_e4m3)
    base[:, :, :, :HW] = shard.reshape(R, CB, P, HW).transpose(2, 0, 1, 3)
    blocks = []
    r0 = 0
    for nr, mode in CHUNKS:
        blk = base[:, r0 : r0 + nr]  # [P, nr, CB, WP]
        if mode == "pe":
            # 8 w-planes, plane-major: [P, 8, nr, CB, 25]
            b2 = blk.reshape(P, nr, CB, 8, OCT).transpose(0, 3, 1, 2, 4)
            blocks.append(b2.reshape(P, nr * SEG))
        else:
            blocks.append(blk.reshape(P, nr * SEG))
        r0 += nr
    return np.ascontiguousarray(np.concatenate(blocks, axis=1))


def _zero_mask():
    mask = np.ones((M, M), dtype=np.float64)
    for i in range(NUM_SAMPLES):
        r = (1 + i) * NUM_NODES
        for c in range(1, M):
            if c % NUM_NODES != 0 and (c - 1) // NUM_NODES != i:
                mask[r, c] = 0.0
    return mask


def _pattern_mixer_np(mat, sigma, lin_w, lin_b, mixed_mat):
    mat = np.asarray(mat, np.float64)
    sigma = np.asarray(sigma, np.float64)
    lin_w = np.asarray(lin_w, np.float64)
    lin_b = np.asarray(lin_b, np.float64)
    mixed_mat = np.asarray(mixed_mat, np.float64)

    T2 = 2 * NUM_FRAME - 1
    dist = np.abs(np.arange(T2, dtype=np.float64) - (NUM_FRAME - 1))
    te = (1.0 / (np.sqrt(2.0 * np.pi) * sigma)) * np.exp(
        -(dist**2) / (2.0 * sigma**2)
    )
    ce = 1.0 / (1.0 + np.exp(-te))
    mixed = (
        np.einsum("hbt,bnm,hb->hntm", ce, mat, lin_w)
        + lin_b[:, None, None, None]
    )
    mixed = np.maximum(mixed, 0.0).reshape(NUM_MIXED, NUM_NODES, T2 * NUM_NODES)
    blocks = [
        mixed[
            :,
            :,
            NUM_NODES * (NUM_SAMPLES - 1 - i) : NUM_NODES * (2 * NUM_SAMPLES - 1 - i),
        ]
        for i in range(NUM_SAMPLES)
    ]
    add_block = np.concatenate(blocks, axis=1)
    mm = mixed_mat.copy()
    mm[:, 1:, 1:] += add_block
    mm *= _zero_mask()[None]
    deg = np.maximum(mm.sum(axis=2), 1.0) ** -0.5
    return (deg[:, :, None] * mm * deg[:, None, :]).astype(np.float32)


def kernel(mat, x, sigma, lin_w, lin_b, mixed_mat, alpha):
    global LAST_RESULT
    x = np.asarray(x, dtype=np.float32).reshape(ROWS_TOTAL, C, HW)
    xq = x.astype(ml_dtypes.float8_e4m3)
    idv = _ident_np()
    in_maps = [
        {"x": _dev_layout(xq[i * R : (i + 1) * R]), "ident": idv}
        for i in range(N_CORES)
    ]
    nc = _get_nc()
    res = run_bass_kernel_spmd(nc, in_maps, core_ids=list(range(N_CORES)))
    LAST_RESULT = res
    norms = np.concatenate([r["out"][0] for r in res.results])
    nrm = norms.reshape(B, M)

    mp = _pattern_mixer_np(mat, sigma, lin_w, lin_b, mixed_mat)
    alpha = np.asarray(alpha, np.float32).reshape(1, NUM_MIXED, 1, 1)
    out = mp[None] + alpha * nrm[:, None, None, :]
    return np.ascontiguousarray(out.astype(np.float32))


# revision 6
# speedup vs baseline: 1.2213x; 1.0057x over previous
"""Trainium2 Bass kernel for nn_Amplified_PatternMixer — fp8 + TensorE fold.

out[b, h, m1, m2] = mixed_pattern[h, m1, m2] + alpha[h] * nrm[b, m2]
  nrm[b, m] = || mean_{hw}(x[b*57+m, :, h, w]) ||_2 over channels

The norm is highly error-tolerant (harness gate: rel_err < 2e-2), so x is
streamed as fp8 e4m3 (measured end-to-end rel err ~1.5e-3), quartering HBM
traffic vs f32.  Per core: 228 rows x 256 ch x 200 hw-padded -> 11.7 MB,
DMA floor ~32.6 us at 358 GB/s.

Layout: channels on partitions — dev[p, ...] holds channel c = b*128 + p.
hw padded 196->200 with zeros.  The reduce work is split three ways so DMA,
TensorE and DVE are all co-critical (~32 us each):
 - 'pe' chunks (20, 190 rows): host lays the chunk out as 8 w-planes; 4 fp8 DoubleRow
   matmuls (stationary = identity pairs, moving AP [p][k=2][cols]) fold the
   8 planes into one PSUM bank (exact f32 adds at ~614 G elem/s), then DVE
   reduces the remaining 25 cols/channel from PSUM.
 - 'dve' chunks (2, 38 rows): plain layout; DVE segmented-reduces 200 cols/channel
   straight from SBUF.
Channel sums -> cs (bf16); DVE squares; a tiny ones-matmul reduces the 128
partitions into PSUM norm^2 contributions (single tail pass).
Tail: combine channel-halves, sqrt (ScalarE, table pre-warmed), DMA
[1, 228] f32 out.  The 57x57 pattern-mixer math runs on host in f64.
"""

import numpy as np
import ml_dtypes

import concourse.bacc as bacc
import concourse.mybir as mybir
import concourse.tile as tile
from concourse.bass_utils import run_bass_kernel_spmd

NUM_BASIC = 5
NUM_MIXED = 4
NUM_FRAME = 8
NUM_NODES = 7
NUM_SAMPLES = 8
M = 1 + NUM_NODES * NUM_FRAME  # 57

N_CORES = 8
B = 32
C = 256
HW = 196          # 14*14
WP = 200          # hw padded to a multiple of 8
ROWS_TOTAL = B * M
R = ROWS_TOTAL // N_CORES   # 228
P = 128
CB = C // P                 # 2
SEG = CB * WP               # 400 per row per partition
FREE = R * SEG              # 91200
OCT = WP // 8               # 25 post-fold cols per (row, half)

# (rows, mode); 'pe' = TensorE 8:1 fold path, 'dve' = direct DVE reduce.
# Measured: PE-path rows ~0.13us/row PE-exec (+0.064 DVE); DVE-path rows
# ~0.42us/row DVE.  190 PE / 38 DVE rows -> DVE ~29us, PE ~24us, both under
# the 32.6us DMA floor.  Long-latency DVE-direct chunks sit early; the tail
# is all fast PE chunks.
CHUNKS = (
    [(10, "pe")] + [(24, "dve")] + [(20, "pe")] * 4
    + [(14, "dve")] + [(20, "pe")] * 4 + [(10, "pe")] + [(6, "pe")] + [(4, "pe")]
)
assert sum(nr for nr, _ in CHUNKS) == R
GROUP_ROWS = 10  # fold-group size within a 'pe' chunk (PSUM bank limit)

LAST_RESULT = None
_NC_CACHE = None


def _build_nc(chunks=None, bufs=10):
    if chunks is None:
        chunks = CHUNKS
    assert sum(nr for nr, _ in chunks) == R
    max_nr = max(nr for nr, _ in chunks)
    f8 = mybir.dt.float8e4
    bf16 = mybir.dt.bfloat16
    f32 = mybir.dt.float32
    DR = mybir.MatmulPerfMode.DoubleRow
    nc = bacc.Bacc(None)
    x = nc.declare_dram_parameter("x", [P, FREE], f8, isOutput=False)
    ident = nc.declare_dram_parameter("ident", [P, 2 * P], f8, isOutput=False)
    out = nc.declare_dram_parameter("out", [1, R], f32, isOutput=True)
    with tile.TileContext(nc) as tc:
        with (
            tc.tile_pool(name="xt_pool", bufs=bufs) as xp,
            tc.tile_pool(name="singles", bufs=1) as sp,
            tc.tile_pool(name="fold", bufs=7, space="PSUM") as fp,
            tc.tile_pool(name="acc", bufs=1, space="PSUM") as ap,
            nc.allow_low_precision(
                reason="fp8 stream + bf16 channel-sums: norm tolerates ~2e-3"
            ),
        ):
            # idt load first IN THE SAME gpsimd queue as the chunk loads:
            # SDMA engines round-robin between queues at packet granularity,
            # so a separate queue does not give low latency — queue-head
            # position does (32KB -> lands in ~0.1us once the ring spins up)
            idt = sp.tile([P, 2 * P], f8, tag="idt")
            nc.gpsimd.dma_start(out=idt[:], in_=ident[:])
            ones = sp.tile([P, 1], bf16, tag="ones")
            nc.vector.memset(ones[:], 1.0)
            lhsT = idt[:].rearrange("p (k m) -> p k m", k=2)
            cs = sp.tile([P, CB * R], bf16, tag="cs")
            sq = sp.tile([P, CB * R], bf16, tag="sq")
            ps2 = ap.tile([1, CB * R], f32, tag="ps2")
            nrm2 = sp.tile([1, R], f32, tag="nrm2")
            nrm = sp.tile([1, R], f32, tag="nrm")
            warm = sp.tile([P, 1], f32, tag="warm")
            nc.scalar.activation(
                warm, ones, mybir.ActivationFunctionType.Sqrt, scale=1.0
            )
            r0 = 0
            for ci, (nr, mode) in enumerate(chunks):
                ns = CB * nr
                c0 = CB * r0
                # single SWDGE queue: chunk completions then pace exactly with
                # cumulative bytes (two queues round-robin at packet level and
                # delay every head-of-queue completion ~2x)
                xt = xp.tile([P, max_nr * SEG], f8, tag="xt")
                eng = nc.gpsimd
                eng.dma_start(
                    out=xt[:, : nr * SEG],
                    in_=x[:, r0 * SEG : (r0 + nr) * SEG],
                )
                if mode == "pe":
                    # one DMA feeds ceil(nr/GROUP_ROWS) fold groups (each
                    # group's PSUM output must fit one bank)
                    g0 = 0
                    while g0 < nr:
                        gr = min(GROUP_ROWS, nr - g0)
                        gs = CB * gr
                        no = gs * OCT
                        ps = fp.tile([P, 512], f32, tag="ps")
                        xv = xt[:, g0 * SEG : (g0 + gr) * SEG].rearrange(
                            "p (a j f) -> p a j f", a=2, j=4
                        )
                        for m in range(4):
                            nc.tensor.matmul(
                                ps[:, :no],
                                lhsT,
                                xv[:, :, m, :],
                                start=(m == 0),
                                stop=(m == 3),
                                perf_mode=DR,
                            )
                        gc = c0 + CB * g0
                        nc.vector.reduce_sum(
                            cs[:, gc : gc + gs],
                            ps[:, :no].rearrange("p (s w) -> p s w", w=OCT),
                            axis=mybir.AxisListType.X,
                        )
                        g0 += gr
                else:
                    nc.vector.reduce_sum(
                        cs[:, c0 : c0 + ns],
                        xt[:, : nr * SEG].rearrange("p (s w) -> p s w", w=WP),
                        axis=mybir.AxisListType.X,
                    )
                r0 += nr
            # tail: one square + one ones-matmul for all 456 cols (in-order
            # engine queues mean any mid-stream flush couples PE to fresh DVE
            # output; a single tail pass avoids all such stalls)
            nc.vector.tensor_mul(sq[:, :], cs[:, :], cs[:, :])
            nc.tensor.matmul(ps2[:, :], ones[:], sq[:, :], start=True, stop=True)
            nc.vector.reduce_sum(
                nrm2[:, :],
                ps2[:, :].rearrange("p (r b) -> p r b", b=CB),
                axis=mybir.AxisListType.X,
            )
            nc.scalar.activation(
                nrm[:, :],
                nrm2[:, :],
                mybir.ActivationFunctionType.Sqrt,
                scale=1.0 / float(HW * HW),
            )
            nc.scalar.dma_start(out=out[:, :], in_=nrm[:, :])
    nc.finalize()
    return nc


def _get_nc():
    global _NC_CACHE
    if _NC_CACHE is None:
        _NC_CACHE = _build_nc()
    return _NC_CACHE


def _ident_np():
    idv = np.zeros((P, 2, P), dtype=ml_dtypes.float8_e4m3)
    for p in range(P):
        idv[p, 0, p] = 1.0
        idv[p, 1, p] = 1.0
    return idv.reshape(P, 2 * P)


def _dev_layout(shard):
    """shard: [R, C, HW] fp8 -> [P, FREE] fp8 device layout."""
    # pad hw and move channels to partitions: [P, R, CB, WP]
    base = np.zeros((P, R, CB, WP), dtype=ml_dtypes.float8_e4m3)
    base[:, :, :, :HW] = shard.reshape(R, CB, P, HW).transpose(2, 0, 1, 3)
    blocks = []
    r0 = 0
    for nr, mode in CHUNKS:
        if mode == "pe":
            # per fold group: 8 w-planes, plane-major [P, 8, gr, CB, 25]
            g0 = 0
            while g0 < nr:
                gr = min(GROUP_ROWS, nr - g0)
                blk = base[:, r0 + g0 : r0 + g0 + gr]
                b2 = blk.reshape(P, gr, CB, 8, OCT).transpose(0, 3, 1, 2, 4)
                blocks.append(b2.reshape(P, gr * SEG))
                g0 += gr
        else:
            blocks.append(base[:, r0 : r0 + nr].reshape(P, nr * SEG))
        r0 += nr
    return np.ascontiguousarray(np.concatenate(blocks, axis=1))


def _zero_mask():
    mask = np.ones((M, M), dtype=np.float64)
    for i in range(NUM_SAMPLES):
        r = (1 + i) * NUM_NODES
        for c in range(1, M):
            if c % NUM_NODES != 0 and (c - 1) // NUM_NODES != i:
                mask[r, c] = 0.0
    return mask


def _pattern_mixer_np(mat, sigma, lin_w, lin_b, mixed_mat):
    mat = np.asarray(mat, np.float64)
    sigma = np.asarray(sigma, np.float64)
    lin_w = np.asarray(lin_w, np.float64)
    lin_b = np.asarray(lin_b, np.float64)
    mixed_mat = np.asarray(mixed_mat, np.float64)

    T2 = 2 * NUM_FRAME - 1
    dist = np.abs(np.arange(T2, dtype=np.float64) - (NUM_FRAME - 1))
    te = (1.0 / (np.sqrt(2.0 * np.pi) * sigma)) * np.exp(
        -(dist**2) / (2.0 * sigma**2)
    )
    ce = 1.0 / (1.0 + np.exp(-te))
    mixed = (
        np.einsum("hbt,bnm,hb->hntm", ce, mat, lin_w)
        + lin_b[:, None, None, None]
    )
    mixed = np.maximum(mixed, 0.0).reshape(NUM_MIXED, NUM_NODES, T2 * NUM_NODES)
    blocks = [
        mixed[
            :,
            :,
            NUM_NODES * (NUM_SAMPLES - 1 - i) : NUM_NODES * (2 * NUM_SAMPLES - 1 - i),
        ]
        for i in range(NUM_SAMPLES)
    ]
    add_block = np.concatenate(blocks, axis=1)
    mm = mixed_mat.copy()
    mm[:, 1:, 1:] += add_block
    mm *= _zero_mask()[None]
    deg = np.maximum(mm.sum(axis=2), 1.0) ** -0.5
    return (deg[:, :, None] * mm * deg[:, None, :]).astype(np.float32)


def kernel(mat, x, sigma, lin_w, lin_b, mixed_mat, alpha):
    global LAST_RESULT
    x = np.asarray(x, dtype=np.float32).reshape(ROWS_TOTAL, C, HW)
    xq = x.astype(ml_dtypes.float8_e4m3)
    idv = _ident_np()
    in_maps = [
        {"x": _dev_layout(xq[i * R : (i + 1) * R]), "ident": idv}
        for i in range(N_CORES)
    ]
    nc = _get_nc()
    res = run_bass_kernel_spmd(nc, in_maps, core_ids=list(range(N_CORES)))
    LAST_RESULT = res
    norms = np.concatenate([r["out"][0] for r in res.results])
    nrm = norms.reshape(B, M)

    mp = _pattern_mixer_np(mat, sigma, lin_w, lin_b, mixed_mat)
    alpha = np.asarray(alpha, np.float32).reshape(1, NUM_MIXED, 1, 1)
    out = mp[None] + alpha * nrm[:, None, None, :]
    return np.ascontiguousarray(out.astype(np.float32))


# revision 7
# speedup vs baseline: 1.2336x; 1.0101x over previous
"""Trainium2 Bass kernel for nn_Amplified_PatternMixer — fp8 + TensorE fold.

out[b, h, m1, m2] = mixed_pattern[h, m1, m2] + alpha[h] * nrm[b, m2]
  nrm[b, m] = || mean_{hw}(x[b*57+m, :, h, w]) ||_2 over channels

The norm is highly error-tolerant (harness gate: rel_err < 2e-2), so x is
streamed as fp8 e4m3 (measured end-to-end rel err ~1.5e-3), quartering HBM
traffic vs f32.  Per core: 228 rows x 256 ch x 200 hw-padded -> 11.7 MB,
DMA floor ~32.6 us at 358 GB/s.

Layout: channels on partitions — dev[p, ...] holds channel c = b*128 + p.
hw padded 196->200 with zeros.  The reduce work is split three ways so DMA,
TensorE and DVE are all co-critical (~32 us each):
 - 'pe' chunks (20, 190 rows): host lays the chunk out as 8 w-planes; 4 fp8 DoubleRow
   matmuls (stationary = identity pairs, moving AP [p][k=2][cols]) fold the
   8 planes into one PSUM bank (exact f32 adds at ~614 G elem/s), then DVE
   reduces the remaining 25 cols/channel from PSUM.
 - 'dve' chunks (2, 38 rows): plain layout; DVE segmented-reduces 200 cols/channel
   straight from SBUF.
Channel sums -> cs (bf16); DVE squares; a tiny ones-matmul reduces the 128
partitions into PSUM norm^2 contributions (single tail pass).
Tail: combine channel-halves, sqrt (ScalarE, table pre-warmed), DMA
[1, 228] f32 out.  The 57x57 pattern-mixer math runs on host in f64.
"""

import numpy as np
import ml_dtypes

import concourse.bacc as bacc
import concourse.mybir as mybir
import concourse.tile as tile
from concourse.bass_utils import run_bass_kernel_spmd

NUM_BASIC = 5
NUM_MIXED = 4
NUM_FRAME = 8
NUM_NODES = 7
NUM_SAMPLES = 8
M = 1 + NUM_NODES * NUM_FRAME  # 57

N_CORES = 8
B = 32
C = 256
HW = 196          # 14*14
WP = 200          # hw padded to a multiple of 8
ROWS_TOTAL = B * M
R = ROWS_TOTAL // N_CORES   # 228
P = 128
CB = C // P                 # 2
SEG = CB * WP               # 400 per row per partition
FREE = R * SEG              # 91200
OCT = WP // 8               # 25 post-fold cols per (row, half)

# (rows, mode); 'pe' = TensorE 8:1 fold path, 'dve' = direct DVE reduce.
# Measured: PE-path rows ~0.13us/row PE-exec (+0.064 DVE); DVE-path rows
# ~0.42us/row DVE.  190 PE / 38 DVE rows -> DVE ~29us, PE ~24us, both under
# the 32.6us DMA floor.  Long-latency DVE-direct chunks sit early; the tail
# is all fast PE chunks.
CHUNKS = (
    [(10, "pe")] + [(24, "dve")] + [(20, "pe")] * 4
    + [(14, "dve")] + [(20, "pe")] * 4 + [(10, "pe")] + [(6, "pe")] + [(4, "pe")]
)
assert sum(nr for nr, _ in CHUNKS) == R
GROUP_ROWS = 10  # fold-group size within a 'pe' chunk (PSUM bank limit)
IDW = 2 * P      # identity-pair weights, folded into the head of x: one
                 # DMA round-trip delivers weights + chunk0 together

LAST_RESULT = None
_NC_CACHE = None


def _build_nc(chunks=None, bufs=10):
    if chunks is None:
        chunks = CHUNKS
    assert sum(nr for nr, _ in chunks) == R
    max_nr = max(nr for nr, _ in chunks)
    f8 = mybir.dt.float8e4
    bf16 = mybir.dt.bfloat16
    f32 = mybir.dt.float32
    DR = mybir.MatmulPerfMode.DoubleRow
    nc = bacc.Bacc(None)
    x = nc.declare_dram_parameter("x", [P, IDW + FREE], f8, isOutput=False)
    out = nc.declare_dram_parameter("out", [1, R], f32, isOutput=True)
    with tile.TileContext(nc) as tc:
        with (
            tc.tile_pool(name="xt_pool", bufs=bufs) as xp,
            tc.tile_pool(name="singles", bufs=1) as sp,
            tc.tile_pool(name="fold", bufs=7, space="PSUM") as fp,
            tc.tile_pool(name="acc", bufs=1, space="PSUM") as ap,
            nc.allow_low_precision(
                reason="fp8 stream + bf16 channel-sums: norm tolerates ~2e-3"
            ),
        ):
            # first DMA delivers identity weights + chunk0 together into a
            # persistent tile (one gen+SWDGE+sem round on the ramp, not two)
            nr0 = chunks[0][0]
            t0 = sp.tile([P, IDW + nr0 * SEG], f8, tag="t0")
            nc.gpsimd.dma_start(out=t0[:], in_=x[:, : IDW + nr0 * SEG])
            ones = sp.tile([P, 1], bf16, tag="ones")
            nc.vector.memset(ones[:], 1.0)
            lhsT = t0[:, :IDW].rearrange("p (k m) -> p k m", k=2)
            cs = sp.tile([P, CB * R], bf16, tag="cs")
            sq = sp.tile([P, CB * R], bf16, tag="sq")
            ps2 = ap.tile([1, CB * R], f32, tag="ps2")
            nrm2 = sp.tile([1, R], f32, tag="nrm2")
            nrm = sp.tile([1, R], f32, tag="nrm")
            warm = sp.tile([P, 1], f32, tag="warm")
            nc.scalar.activation(
                warm, ones, mybir.ActivationFunctionType.Sqrt, scale=1.0
            )
            r0 = 0
            for ci, (nr, mode) in enumerate(chunks):
                ns = CB * nr
                c0 = CB * r0
                if ci == 0:
                    data = t0[:, IDW : IDW + nr * SEG]
                else:
                    # single SWDGE queue: chunk completions pace exactly with
                    # cumulative bytes (two queues round-robin at packet level
                    # and delay every head-of-queue completion ~2x)
                    xt = xp.tile([P, max_nr * SEG], f8, tag="xt")
                    nc.gpsimd.dma_start(
                        out=xt[:, : nr * SEG],
                        in_=x[:, IDW + r0 * SEG : IDW + (r0 + nr) * SEG],
                    )
                    data = xt[:, : nr * SEG]
                if mode == "pe":
                    # one DMA feeds ceil(nr/GROUP_ROWS) fold groups (each
                    # group's PSUM output must fit one bank)
                    g0 = 0
                    while g0 < nr:
                        gr = min(GROUP_ROWS, nr - g0)
                        gs = CB * gr
                        no = gs * OCT
                        ps = fp.tile([P, 512], f32, tag="ps")
                        xv = data[:, g0 * SEG : (g0 + gr) * SEG].rearrange(
                            "p (a j f) -> p a j f", a=2, j=4
                        )
                        for m in range(4):
                            nc.tensor.matmul(
                                ps[:, :no],
                                lhsT,
                                xv[:, :, m, :],
                                start=(m == 0),
                                stop=(m == 3),
                                perf_mode=DR,
                            )
                        gc = c0 + CB * g0
                        nc.vector.reduce_sum(
                            cs[:, gc : gc + gs],
                            ps[:, :no].rearrange("p (s w) -> p s w", w=OCT),
                            axis=mybir.AxisListType.X,
                        )
                        g0 += gr
                else:
                    nc.vector.reduce_sum(
                        cs[:, c0 : c0 + ns],
                        data.rearrange("p (s w) -> p s w", w=WP),
                        axis=mybir.AxisListType.X,
                    )
                r0 += nr
            # tail: one square + one ones-matmul for all 456 cols (in-order
            # engine queues mean any mid-stream flush couples PE to fresh DVE
            # output; a single tail pass avoids all such stalls)
            nc.vector.tensor_mul(sq[:, :], cs[:, :], cs[:, :])
            nc.tensor.matmul(ps2[:, :], ones[:], sq[:, :], start=True, stop=True)
            nc.vector.reduce_sum(
                nrm2[:, :],
                ps2[:, :].rearrange("p (r b) -> p r b", b=CB),
                axis=mybir.AxisListType.X,
            )
            nc.scalar.activation(
                nrm[:, :],
                nrm2[:, :],
                mybir.ActivationFunctionType.Sqrt,
                scale=1.0 / float(HW * HW),
            )
            nc.sync.dma_start(out=out[:, :], in_=nrm[:, :])
    nc.finalize()
    return nc


def _get_nc():
    global _NC_CACHE
    if _NC_CACHE is None:
        _NC_CACHE = _build_nc()
    return _NC_CACHE


def _ident_np():
    idv = np.zeros((P, 2, P), dtype=ml_dtypes.float8_e4m3)
    for p in range(P):
        idv[p, 0, p] = 1.0
        idv[p, 1, p] = 1.0
    return idv.reshape(P, 2 * P)


def _dev_layout(shard):
    """shard: [R, C, HW] fp8 -> [P, IDW + FREE] fp8 device layout
    (identity-pair weights prepended)."""
    # pad hw and move channels to partitions: [P, R, CB, WP]
    base = np.zeros((P, R, CB, WP), dtype=ml_dtypes.float8_e4m3)
    base[:, :, :, :HW] = shard.reshape(R, CB, P, HW).transpose(2, 0, 1, 3)
    blocks = [_ident_np()]
    r0 = 0
    for nr, mode in CHUNKS:
        if mode == "pe":
            # per fold group: 8 w-planes, plane-major [P, 8, gr, CB, 25]
            g0 = 0
            while g0 < nr:
                gr = min(GROUP_ROWS, nr - g0)
                blk = base[:, r0 + g0 : r0 + g0 + gr]
                b2 = blk.reshape(P, gr, CB, 8, OCT).transpose(0, 3, 1, 2, 4)
                blocks.append(b2.reshape(P, gr * SEG))
                g0 += gr
        else:
            blocks.append(base[:, r0 : r0 + nr].reshape(P, nr * SEG))
        r0 += nr
    return np.ascontiguousarray(np.concatenate(blocks, axis=1))


def _zero_mask():
    mask = np.ones((M, M), dtype=np.float64)
    for i in range(NUM_SAMPLES):
        r = (1 + i) * NUM_NODES
        for c in range(1, M):
            if c % NUM_NODES != 0 and (c - 1) // NUM_NODES != i:
                mask[r, c] = 0.0
    return mask


def _pattern_mixer_np(mat, sigma, lin_w, lin_b, mixed_mat):
    mat = np.asarray(mat, np.float64)
    sigma = np.asarray(sigma, np.float64)
    lin_w = np.asarray(lin_w, np.float64)
    lin_b = np.asarray(lin_b, np.float64)
    mixed_mat = np.asarray(mixed_mat, np.float64)

    T2 = 2 * NUM_FRAME - 1
    dist = np.abs(np.arange(T2, dtype=np.float64) - (NUM_FRAME - 1))
    te = (1.0 / (np.sqrt(2.0 * np.pi) * sigma)) * np.exp(
        -(dist**2) / (2.0 * sigma**2)
    )
    ce = 1.0 / (1.0 + np.exp(-te))
    mixed = (
        np.einsum("hbt,bnm,hb->hntm", ce, mat, lin_w)
        + lin_b[:, None, None, None]
    )
    mixed = np.maximum(mixed, 0.0).reshape(NUM_MIXED, NUM_NODES, T2 * NUM_NODES)
    blocks = [
        mixed[
            :,
            :,
            NUM_NODES * (NUM_SAMPLES - 1 - i) : NUM_NODES * (2 * NUM_SAMPLES - 1 - i),
        ]
        for i in range(NUM_SAMPLES)
    ]
    add_block = np.concatenate(blocks, axis=1)
    mm = mixed_mat.copy()
    mm[:, 1:, 1:] += add_block
    mm *= _zero_mask()[None]
    deg = np.maximum(mm.sum(axis=2), 1.0) ** -0.5
    return (deg[:, :, None] * mm * deg[:, None, :]).astype(np.float32)


def kernel(mat, x, sigma, lin_w, lin_b, mixed_mat, alpha):
    global LAST_RESULT
    x = np.asarray(x, dtype=np.float32).reshape(ROWS_TOTAL, C, HW)
    xq = x.astype(ml_dtypes.float8_e4m3)
    in_maps = [
        {"x": _dev_layout(xq[i * R : (i + 1) * R])} for i in range(N_CORES)
    ]
    nc = _get_nc()
    res = run_bass_kernel_spmd(nc, in_maps, core_ids=list(range(N_CORES)))
    LAST_RESULT = res
    norms = np.concatenate([r["out"][0] for r in res.results])
    nrm = norms.reshape(B, M)

    mp = _pattern_mixer_np(mat, sigma, lin_w, lin_b, mixed_mat)
    alpha = np.asarray(alpha, np.float32).reshape(1, NUM_MIXED, 1, 1)
    out = mp[None] + alpha * nrm[:, None, None, :]
    return np.ascontiguousarray(out.astype(np.float32))
